# revision 39
# baseline (speedup 1.0000x reference)
"""Tropical min-max matmul kernel for Trainium2.

out[b, o] = min_i max(x[b, i], weight[i, o])   with  x: [1024, 512], weight: [512, 512], fp32.

Strategy
--------
Data-parallel over the batch dim: 8 NeuronCores x 128 rows of x each; weight
replicated (no collectives). Per core, the weight is held transposed
(wT[o, i], o on partitions in 4 row-blocks) so the contraction axis i is the
DVE free axis, and batch rows are processed in groups of 16 with three fat
instructions per group:

  1. A DMA whose source access pattern has partition stride 0 broadcasts the
     group's x rows across all 128 partitions (SBUF bc tile, double-buffered,
     two groups loaded per DMA).
  2. One wide DVE tensor_tensor(max) over [128, 16*4*512] computes
     max(wT[o', i], x[b, i]) for every (b in group, o-block, i) — the
     weight view repeats via a stride-0 dim, the bc view likewise; 32768
     free elements per instruction (the ISA num_elem field caps at 65535,
     and the fp32 scratch at 128KB/partition is the SBUF limit).
  3. One DVE tensor_reduce(min, axis=X) over the scratch viewed as
     [128, 16, 4, 512] finishes min over i, writing the [128, 16*4] result
     with a strided AP directly into the output tile.

The per-core result lands as ot[128, 4*128] = [o-within-block, block*128+b];
the host reassembles it into out[b, o]. Exact fp32 (min/max select values,
so the result is bit-identical to the reference).

This shape is chosen to be near-optimal both on real silicon (DVE-bound at
~550us/core by the calibrated cost model: TT and reduce both run 1x fp32 at
~1 elem/cycle/lane) and in instruction-dispatch-bound runtimes (only ~30
instructions per core).
"""

import os
import sys

for _p in ("/opt/trn_rl_repo", "/root/.axon_site/_ro/trn_rl_repo"):
    if os.path.isdir(_p) and _p not in sys.path:
        sys.path.insert(0, _p)

import numpy as np

import concourse.bass as bass
import concourse.mybir as mybir
from concourse.bass_utils import run_bass_kernel_spmd

B, I, O = 1024, 512, 512
NCORES = 8
BS = B // NCORES   # 128 batch rows per core
NCH = I // 128     # 4 i-chunks
OBLK = O // 128    # 4 output-feature blocks

# Flipped to True by test.py to collect an NTFF profile; results stashed in
# LAST_RESULTS for inspection.
TRACE = False
LAST_RESULTS = None
# When > 0, kernel() reruns the SPMD executable this many extra times and
# records per-run wall times (seconds) in BENCH_TIMES.
BENCH = 0
BENCH_TIMES = None

_F32 = mybir.dt.float32
_F16 = mybir.dt.float16

# "fp32" (exact) or "fp16" (faster DVE modes, ~1e-4 rel err)
DTYPE_MODE = os.environ.get("MINMAX_DTYPE", "fp32")


def _build_nc_wide(dt, detect_races=True, repeat=1, group=16):
    """Wide-group variant: GROUP batch rows per step, three fat instructions
    per group (DMA partition-broadcast of x rows; one wide tensor_tensor max
    over [128, GROUP*4*512]; one strided tensor_reduce min). Minimizes
    instruction count while staying near the DVE roofline.

    Needs wT = weight.T from the host: tiles wT_t[o', i] put o' on partitions
    so the i axis is free (reduce axis); x rows broadcast across partitions.
    """
    nc = bass.Bass(detect_race_conditions=detect_races)
    G = group
    NGRP = BS // G
    W = G * NCH * I  # wide op free size per group

    xd = nc.declare_dram_parameter("x", [BS, I], dt, isOutput=False)
    wt_d = nc.declare_dram_parameter("wT", [O, I], dt, isOutput=False)
    out_d = nc.declare_dram_parameter("ot", [128, OBLK * BS], dt, isOutput=True)

    x_rows = xd.rearrange("(g r) i -> g (r i)", r=G)  # [NGRP, G*I]

    with (
        nc.sbuf_tensor([128, OBLK * I], dt) as wt_sb,
        nc.sbuf_tensor([128, 2 * G * I], dt) as bc_sb,   # double-buffered bcast
        nc.sbuf_tensor([128, W], dt) as scr_sb,
        nc.sbuf_tensor([128, OBLK * BS], dt) as ot_sb,
        nc.semaphore("dma_sem") as dma_sem,
        nc.semaphore("v_sem") as v_sem,
        nc.Block() as block,
    ):
        NB = repeat * NGRP

        def bc_tile(g):
            j = g % 2
            return bc_sb[:, j * G * I:(j + 1) * G * I]

        @block.sync
        def _(sync):
            sync.dma_start(
                out=wt_sb[:, :].rearrange("p (t i) -> p t i", t=OBLK),
                in_=wt_d.rearrange("(t p) i -> p t i", p=128),
            ).then_inc(dma_sem, 16)
            # broadcast x rows two groups at a time (one DMA fills both
            # halves of the double buffer)
            n_pair_dma = 0
            for gg in range(0, NB, 2):
                g = gg % NGRP
                if gg >= 2:
                    # both halves consumed by the TTs of gg-2 and gg-1
                    sync.wait_ge(v_sem, 2 * gg - 1)
                src = x_rows[g:g + 2, :]
                src_b = bass.AP(
                    tensor=src.tensor,
                    offset=src.offset,
                    ap=[[0, 128], [G * I, 2], [1, G * I]],
                )
                sync.dma_start(out=bc_sb[:, :], in_=src_b).then_inc(dma_sem, 16)
                n_pair_dma += 1
            sync.wait_ge(v_sem, 2 * NB)
            sync.dma_start(out=out_d[:, :], in_=ot_sb[:, :]).then_inc(dma_sem, 16)
            sync.wait_ge(dma_sem, 16 * (n_pair_dma + 2))

        @block.vector
        def _(vector):
            wt_v = wt_sb[:, :]
            scr_v = scr_sb[:, :]
            for gg in range(NB):
                g = gg % NGRP
                if gg % 2 == 0:
                    vector.wait_ge(dma_sem, 16 * (gg // 2 + 2))
                bc = bc_tile(gg)
                in0 = bass.AP(
                    tensor=wt_v.tensor, offset=wt_v.offset,
                    ap=[[wt_v.ap[0][0], 128], [0, G], [I, OBLK], [1, I]],
                )
                in1 = bass.AP(
                    tensor=bc.tensor, offset=bc.offset,
                    ap=[[bc.ap[0][0], 128], [I, G], [0, OBLK], [1, I]],
                )
                out = bass.AP(
                    tensor=scr_v.tensor, offset=scr_v.offset,
                    ap=[[scr_v.ap[0][0], 128], [OBLK * I, G], [I, OBLK], [1, I]],
                )
                nc.vector.tensor_tensor(
                    out=out, in0=in0, in1=in1, op=mybir.AluOpType.max
                ).then_inc(v_sem, 1)
                ot_ap = ot_sb[:, :]
                red_out = bass.AP(
                    tensor=ot_ap.tensor,
                    offset=ot_ap.offset + g * G,
                    ap=[[ot_ap.ap[0][0], 128], [1, G], [BS, OBLK]],
                )
                nc.vector.tensor_reduce(
                    out=red_out,
                    in_=out,
                    op=mybir.AluOpType.min,
                    axis=mybir.AxisListType.X,
                ).then_inc(v_sem, 1)

    return nc


_NC_CACHE = {}


def _get_nc(mode):
    if mode not in _NC_CACHE:
        if mode == "fp16":
            _NC_CACHE[mode] = _build_nc_wide(_F16, group=16)
        else:
            _NC_CACHE[mode] = _build_nc_wide(_F32, group=16)
    return _NC_CACHE[mode]


def kernel(x, weight):
    global LAST_RESULTS
    x = np.asarray(x)
    weight = np.asarray(weight)
    in_dtype = x.dtype

    mode = DTYPE_MODE
    npdt = np.float16 if mode == "fp16" else np.float32
    nc = _get_nc(mode)

    wt_h = np.ascontiguousarray(weight.T.astype(npdt))  # [O, I]
    xh = x.astype(npdt)
    in_maps = [
        {
            "x": np.ascontiguousarray(xh[c * BS:(c + 1) * BS]),
            "wT": wt_h,
        }
        for c in range(NCORES)
    ]

    res = run_bass_kernel_spmd(nc, in_maps, list(range(NCORES)), trace=TRACE)
    LAST_RESULTS = res

    if BENCH > 0:
        import time as _time

        global BENCH_TIMES
        BENCH_TIMES = []
        for _ in range(BENCH):
            t0 = _time.perf_counter()
            run_bass_kernel_spmd(nc, in_maps, list(range(NCORES)), trace=False)
            BENCH_TIMES.append(_time.perf_counter() - t0)

    # ot[oo, t*BS + b] = out_core[b, t*128 + oo]
    parts = []
    for c in range(NCORES):
        ot = np.asarray(res.results[c]["ot"])          # [128, OBLK*BS]
        oc = ot.reshape(128, OBLK, BS).transpose(2, 1, 0).reshape(BS, O)
        parts.append(oc)
    out = np.concatenate(parts, axis=0)
    return out.astype(in_dtype)
